# revision 1
# baseline (speedup 1.0000x reference)
"""DAGCN layer kernel for 8 Trainium2 NeuronCores (Bass/Tile, SPMD).

Math (equal to the reference by linearity of the edge MLP):
  hs = h @ W_src ; ht = h @ W_tgt
  agg[n] = (sum_{e:dst=n} hs[src[e]] + deg[n]*(ht[n] + b_src + b_tgt)) / max(deg[n],1)
  then multi-head attention (q from h rows, k/v from agg) + FFN with LayerNorms.

Sharding: edges sorted by dst, bucketed per (core, 128-node block), padded to a
fixed tile count; core c owns dst/query rows [512c, 512c+512). Per-edge work is
an indirect-DMA gather of hs-table rows + one-hot matmul scatter into PSUM.
agg slices are AllGathered (bf16, transposed) so each core holds full k/v.
"""

import contextlib
import numpy as np
import ml_dtypes

import concourse.bass as bass
import concourse.bacc as bacc
import concourse.tile as tile
from concourse import mybir
from concourse.bass_utils import run_bass_kernel_spmd
from concourse.masks import make_identity

N, H, HEADS, E = 4096, 256, 4, 262144
DH = H // HEADS          # 64
NCORES = 8
RPC = N // NCORES        # 512 nodes per core
NBLK = RPC // 128        # 4 dst blocks per core
TPB = 68                 # tiles per block (68*128 = 8704 padded edges per block)
NT = NBLK * TPB          # 272 edge tiles per core
D = H + 2                # table row: 256 features + ones col + pad
TBL_ROWS = N + 128
EPS = 1e-5
NCH = N // 128           # 32
HCH = H // 128           # 2

BF = mybir.dt.bfloat16
F32 = mybir.dt.float32
I32 = mybir.dt.int32
AF = mybir.ActivationFunctionType

_CACHE = {}


def _layernorm_rows(nc, pool, z, out_t, g, be, i, nm, epst=None):
    """LayerNorm along the free dim of a [128, H] f32 row tile."""
    stats = pool.tile([128, 6], F32, name=f"{nm}_st{i}", tag=f"{nm}_st")
    nc.vector.bn_stats(out=stats[:], in_=z[:])
    mv = pool.tile([128, 2], F32, name=f"{nm}_mv{i}", tag=f"{nm}_mv")
    nc.vector.bn_aggr(out=mv[:], in_=stats[:])
    sd = pool.tile([128, 1], F32, name=f"{nm}_sd{i}", tag=f"{nm}_sd")
    nc.scalar.activation(out=sd[:], in_=mv[:, 1:2], func=AF.Sqrt,
                         bias=epst[:, 0:1], scale=1.0)
    rstd = pool.tile([128, 1], F32, name=f"{nm}_rs{i}", tag=f"{nm}_rs")
    nc.vector.reciprocal(out=rstd[:], in_=sd[:])
    nmu = pool.tile([128, 1], F32, name=f"{nm}_nm{i}", tag=f"{nm}_nm")
    nc.vector.tensor_tensor(out=nmu[:], in0=mv[:, 0:1], in1=rstd[:],
                            op=mybir.AluOpType.mult)
    nc.vector.tensor_scalar_mul(nmu[:], nmu[:], -1.0)
    zn = pool.tile([128, z.shape[1]], F32, name=f"{nm}_zn{i}", tag=f"{nm}_zn")
    nc.scalar.activation(out=zn[:], in_=z[:], func=AF.Identity,
                         bias=nmu[:, 0:1], scale=rstd[:, 0:1])
    nc.vector.tensor_tensor(out=zn[:], in0=zn[:], in1=g[:], op=mybir.AluOpType.mult)
    nc.vector.tensor_add(out_t[:], zn[:], be[:])


def _build_program():
    nc = bacc.Bacc("TRN2", target_bir_lowering=False, debug=False, num_devices=NCORES)

    hT_bf = nc.dram_tensor("hT_bf", [H, N], BF, kind="ExternalInput")
    hT_own_d = nc.dram_tensor("hT_own", [H, RPC], BF, kind="ExternalInput")
    h_rows = nc.dram_tensor("h_rows", [RPC, H], F32, kind="ExternalInput")
    srcT = nc.dram_tensor("srcT", [128, NT], I32, kind="ExternalInput")
    dstT = nc.dram_tensor("dstT", [128, NT], I32, kind="ExternalInput")
    w_src = nc.dram_tensor("w_src", [H, H], BF, kind="ExternalInput")
    w_tgt = nc.dram_tensor("w_tgt", [H, H], BF, kind="ExternalInput")
    w_q = nc.dram_tensor("w_q", [H, H], BF, kind="ExternalInput")
    w_k = nc.dram_tensor("w_k", [H, H], BF, kind="ExternalInput")
    w_v = nc.dram_tensor("w_v", [H, H], BF, kind="ExternalInput")
    w_o = nc.dram_tensor("w_o", [H, H], BF, kind="ExternalInput")
    w_1 = nc.dram_tensor("w_1", [H, 2 * H], BF, kind="ExternalInput")
    w_2 = nc.dram_tensor("w_2", [2 * H, H], BF, kind="ExternalInput")
    bst_b = nc.dram_tensor("bst_b", [128, H], F32, kind="ExternalInput")
    bq_c = nc.dram_tensor("bq_c", [128, HCH], F32, kind="ExternalInput")
    bk_c = nc.dram_tensor("bk_c", [128, HCH], F32, kind="ExternalInput")
    bv_b = nc.dram_tensor("bv_b", [128, H], F32, kind="ExternalInput")
    bo_b = nc.dram_tensor("bo_b", [128, H], F32, kind="ExternalInput")
    b1_c = nc.dram_tensor("b1_c", [128, 4], F32, kind="ExternalInput")
    b2_b = nc.dram_tensor("b2_b", [128, H], F32, kind="ExternalInput")
    g1_b = nc.dram_tensor("g1_b", [128, H], F32, kind="ExternalInput")
    be1_b = nc.dram_tensor("be1_b", [128, H], F32, kind="ExternalInput")
    g2_b = nc.dram_tensor("g2_b", [128, H], F32, kind="ExternalInput")
    be2_b = nc.dram_tensor("be2_b", [128, H], F32, kind="ExternalInput")
    out = nc.dram_tensor("out", [RPC, H], F32, kind="ExternalOutput")
    table = nc.dram_tensor("hs_table", [TBL_ROWS, D], BF)

    with tile.TileContext(nc) as tc, contextlib.ExitStack() as ctx:
        singles = ctx.enter_context(tc.tile_pool(name="singles", bufs=1))
        wpool = ctx.enter_context(tc.tile_pool(name="wpool", bufs=1))
        hs_sb = ctx.enter_context(tc.tile_pool(name="hs_sb", bufs=4))
        gpool = ctx.enter_context(tc.tile_pool(name="gpool", bufs=16))
        ohpool = ctx.enter_context(tc.tile_pool(name="ohpool", bufs=8))
        epool = ctx.enter_context(tc.tile_pool(name="epool", bufs=8))

        # ---------- constants ----------
        hT = [singles.tile([128, N], BF, name=f"hT{j}") for j in range(HCH)]
        for j in range(HCH):
            nc.sync.dma_start(out=hT[j][:], in_=hT_bf[j * 128:(j + 1) * 128, :])
        hTo = [singles.tile([128, RPC], BF, name=f"hTo{j}") for j in range(HCH)]
        for j in range(HCH):
            nc.sync.dma_start(out=hTo[j][:], in_=hT_own_d[j * 128:(j + 1) * 128, :])

        def load_w(t, name, rows, cols):
            w = [wpool.tile([128, cols], BF, name=f"{name}{i}") for i in range(rows // 128)]
            for i in range(rows // 128):
                nc.sync.dma_start(out=w[i][:], in_=t[i * 128:(i + 1) * 128, :])
            return w

        Wsrc = load_w(w_src, "Wsrc", H, H)
        Wtgt = load_w(w_tgt, "Wtgt", H, H)
        Wq = load_w(w_q, "Wq", H, H)
        Wk = load_w(w_k, "Wk", H, H)
        Wv = load_w(w_v, "Wv", H, H)
        Wo = load_w(w_o, "Wo", H, H)
        W1 = load_w(w_1, "W1", H, 2 * H)
        W2 = load_w(w_2, "W2", 2 * H, H)

        def load_b(t, name, shape):
            b = singles.tile(list(shape), F32, name=name)
            nc.sync.dma_start(out=b[:], in_=t[:])
            return b

        bstb = load_b(bst_b, "bstb", (128, H))
        bqc = load_b(bq_c, "bqc", (128, HCH))
        bkc = load_b(bk_c, "bkc", (128, HCH))
        bvb = load_b(bv_b, "bvb", (128, H))
        bob = load_b(bo_b, "bob", (128, H))
        b1c = load_b(b1_c, "b1c", (128, 4))
        b2b = load_b(b2_b, "b2b", (128, H))
        g1b = load_b(g1_b, "g1b", (128, H))
        be1b = load_b(be1_b, "be1b", (128, H))
        g2b = load_b(g2_b, "g2b", (128, H))
        be2b = load_b(be2_b, "be2b", (128, H))

        src_t = singles.tile([128, NT], I32)
        nc.sync.dma_start(out=src_t[:], in_=srcT[:])
        dst_raw = singles.tile([128, NT], I32)
        nc.sync.dma_start(out=dst_raw[:], in_=dstT[:])
        dst_t = singles.tile([128, NT], F32)
        nc.vector.tensor_copy(out=dst_t[:], in_=dst_raw[:])
        iotaf = []
        for b in range(NBLK):
            it_i = singles.tile([128, 128], I32, name=f"iotai{b}")
            nc.gpsimd.iota(it_i[:], pattern=[[1, 128]], base=b * 128, channel_multiplier=0)
            it_f = singles.tile([128, 128], F32, name=f"iotaf{b}")
            nc.vector.tensor_copy(out=it_f[:], in_=it_i[:])
            iotaf.append(it_f)
        hrows = [singles.tile([128, H], F32, name=f"hrows{i}") for i in range(NBLK)]
        for i in range(NBLK):
            nc.sync.dma_start(out=hrows[i][:], in_=h_rows[i * 128:(i + 1) * 128, :])
        ident = singles.tile([128, 128], F32)
        make_identity(nc, ident[:])
        epst = singles.tile([128, 1], F32)
        nc.vector.memset(epst[:], EPS)

        # ---------- phase 1: hs table, ht rows, qT ----------
        ph1 = tc.tile_pool(name="ph1_ps", bufs=2, space="PSUM")
        hs_ps = mid_ps = ph1.__enter__()
        for nch in range(NCH):
            ps = hs_ps.tile([128, H], F32)
            for k in range(HCH):
                nc.tensor.matmul(out=ps[:], lhsT=hT[k][:, nch * 128:(nch + 1) * 128],
                                 rhs=Wsrc[k][:], start=(k == 0), stop=(k == HCH - 1))
            row = hs_sb.tile([128, D], BF)
            nc.scalar.copy(out=row[:, 0:H], in_=ps[:])
            nc.vector.memset(row[:, H:H + 1], 1.0)
            nc.vector.memset(row[:, H + 1:D], 0.0)
            nc.sync.dma_start(out=table[nch * 128:(nch + 1) * 128, :], in_=row[:])
        zrow = singles.tile([128, D], BF)
        nc.vector.memset(zrow[:], 0.0)
        nc.sync.dma_start(out=table[N:N + 128, :], in_=zrow[:])

        htr = [singles.tile([128, H], F32, name=f"htr{i}") for i in range(NBLK)]
        for i in range(NBLK):
            ps = mid_ps.tile([128, H], F32)
            for k in range(HCH):
                nc.tensor.matmul(out=ps[:], lhsT=hTo[k][:, i * 128:(i + 1) * 128],
                                 rhs=Wtgt[k][:], start=(k == 0), stop=(k == HCH - 1))
            nc.vector.tensor_copy(out=htr[i][:], in_=ps[:])

        qT = [singles.tile([128, RPC], BF, name=f"qT{j}") for j in range(HCH)]
        for j in range(HCH):
            ps = mid_ps.tile([128, RPC], F32)
            for k in range(HCH):
                nc.tensor.matmul(out=ps[:], lhsT=Wq[k][:, j * 128:(j + 1) * 128],
                                 rhs=hTo[k][:], start=(k == 0), stop=(k == HCH - 1))
            nc.scalar.activation(out=qT[j][:], in_=ps[:], func=AF.Identity,
                                 bias=bqc[:, j:j + 1], scale=1.0)

        ph1.__exit__(None, None, None)

        # ---------- phase 2: gather + one-hot scatter ----------
        ph2 = tc.tile_pool(name="agg_ps", bufs=1, space="PSUM")
        agg_ps = ph2.__enter__()
        aggp = [agg_ps.tile([128, D], F32, name=f"aggp{b}") for b in range(NBLK)]
        for t in range(NT):
            b = t // TPB
            g = gpool.tile([128, D], BF)
            nc.gpsimd.indirect_dma_start(
                out=g[:], out_offset=None, in_=table[:],
                in_offset=bass.IndirectOffsetOnAxis(ap=src_t[:, t:t + 1], axis=0))
            oh = ohpool.tile([128, 128], BF)
            nc.vector.tensor_tensor(
                out=oh[:], in0=dst_t[:, t:t + 1].to_broadcast([128, 128]),
                in1=iotaf[b][:], op=mybir.AluOpType.is_equal)
            nc.tensor.matmul(out=aggp[b][:], lhsT=oh[:], rhs=g[:],
                             start=(t % TPB == 0), stop=(t % TPB == TPB - 1))

        # finalize agg rows
        agg_rows = [singles.tile([128, H], F32, name=f"aggrow{b}") for b in range(NBLK)]
        for b in range(NBLK):
            deg = singles.tile([128, 1], F32, name=f"deg{b}")
            nc.vector.tensor_copy(out=deg[:], in_=aggp[b][:, H:H + 1])
            tmp = singles.tile([128, H], F32, name=f"fin_tmp{b}", tag="fin_tmp")
            nc.vector.tensor_add(tmp[:], htr[b][:], bstb[:])
            nc.vector.tensor_tensor(out=tmp[:], in0=tmp[:],
                                    in1=deg[:, 0:1].to_broadcast([128, H]),
                                    op=mybir.AluOpType.mult)
            nc.vector.tensor_add(tmp[:], tmp[:], aggp[b][:, 0:H])
            degc = singles.tile([128, 1], F32, name=f"degc{b}", tag="degc")
            nc.vector.tensor_scalar_max(degc[:], deg[:], 1.0)
            rec = singles.tile([128, 1], F32, name=f"rec{b}", tag="recb")
            nc.vector.reciprocal(out=rec[:], in_=degc[:])
            nc.vector.tensor_tensor(out=agg_rows[b][:], in0=tmp[:],
                                    in1=rec[:, 0:1].to_broadcast([128, H]),
                                    op=mybir.AluOpType.mult)

        ph2.__exit__(None, None, None)
        ph3 = tc.tile_pool(name="tp_ps", bufs=2, space="PSUM")
        tp_ps = ph3.__enter__()
        aggT_own = [singles.tile([128, RPC], BF, name=f"aggTo{j}") for j in range(HCH)]
        for b in range(NBLK):
            for j in range(HCH):
                tp = tp_ps.tile([128, 128], F32, name=f"tp_{b}_{j}", tag="tp")
                nc.tensor.transpose(out=tp[:], in_=agg_rows[b][:, j * 128:(j + 1) * 128],
                                    identity=ident[:])
                nc.vector.tensor_copy(out=aggT_own[j][:, b * 128:(b + 1) * 128], in_=tp[:])

        ph3.__exit__(None, None, None)

        # ---------- phase 3: AllGather ----------
        with tc.tile_pool(name="dram", bufs=1, space="DRAM") as dram:
            cc_in = dram.tile([H, RPC], BF)
            cc_out = dram.tile([NCORES * H, RPC], BF, addr_space="Shared")
            for j in range(HCH):
                nc.gpsimd.dma_start(out=cc_in[j * 128:(j + 1) * 128, :], in_=aggT_own[j][:])
            nc.gpsimd.collective_compute(
                "AllGather", mybir.AluOpType.bypass,
                replica_groups=[list(range(NCORES))],
                ins=[cc_in.opt()], outs=[cc_out.opt()])
            aggT_full = [singles.tile([128, N], BF, name=f"aggTf{j}") for j in range(HCH)]
            ccv = cc_out[:].rearrange("(c h) f -> c h f", c=NCORES)
            for j in range(HCH):
                for c in range(NCORES):
                    nc.sync.dma_start(out=aggT_full[j][:, c * RPC:(c + 1) * RPC],
                                      in_=ccv[c, j * 128:(j + 1) * 128, :])

            # ---------- phase 4: kT and v_ext ----------
            ph4 = tc.tile_pool(name="kv_ps", bufs=2, space="PSUM")
            mid_ps = ph4.__enter__()
            kT = [singles.tile([128, N], BF, name=f"kT{j}") for j in range(HCH)]
            for j in range(HCH):
                for piece in range(N // 512):
                    ps = mid_ps.tile([128, 512], F32)
                    for k in range(HCH):
                        nc.tensor.matmul(
                            out=ps[:], lhsT=Wk[k][:, j * 128:(j + 1) * 128],
                            rhs=aggT_full[k][:, piece * 512:(piece + 1) * 512],
                            start=(k == 0), stop=(k == HCH - 1))
                    nc.scalar.activation(out=kT[j][:, piece * 512:(piece + 1) * 512],
                                         in_=ps[:], func=AF.Identity,
                                         bias=bkc[:, j:j + 1], scale=1.0)
            vext = [singles.tile([128, HEADS * (DH + 1)], BF, name=f"vext{kc}")
                    for kc in range(NCH)]
            for kc in range(NCH):
                ps = mid_ps.tile([128, H], F32)
                for k in range(HCH):
                    nc.tensor.matmul(out=ps[:], lhsT=aggT_full[k][:, kc * 128:(kc + 1) * 128],
                                     rhs=Wv[k][:], start=(k == 0), stop=(k == HCH - 1))
                vtmp = singles.tile([128, H], F32, name=f"vtmp{kc}", tag="vtmp")
                nc.vector.tensor_add(vtmp[:], ps[:], bvb[:])
                for h in range(HEADS):
                    nc.vector.tensor_copy(out=vext[kc][:, h * (DH + 1):h * (DH + 1) + DH],
                                          in_=vtmp[:, h * DH:(h + 1) * DH])
                    nc.vector.memset(vext[kc][:, h * (DH + 1) + DH:(h + 1) * (DH + 1)], 1.0)

            ph4.__exit__(None, None, None)

            # ---------- phase 5: attention ----------
            SCALE = float(1.0 / np.sqrt(DH))
            ph5c = tc.tile_pool(name="ctx_ps", bufs=1, space="PSUM")
            ctx_ps = ph5c.__enter__()
            ph5q = tc.tile_pool(name="qk_ps", bufs=2, space="PSUM")
            qk_ps = ph5q.__enter__()
            ctxp = [ctx_ps.tile([DH + 1, RPC], F32, name=f"ctxp{h}") for h in range(HEADS)]
            for kc in range(NCH):
                es = []
                for h in range(HEADS):
                    j, r = h // 2, (h % 2) * 64
                    ps_s = qk_ps.tile([128, RPC], F32, name=f"ps_s{h}_{kc}",
                                      tag=f"ps_s{h % 2}")
                    nc.tensor.matmul(out=ps_s[:],
                                     lhsT=kT[j][r:r + 64, kc * 128:(kc + 1) * 128],
                                     rhs=qT[j][r:r + 64, :], start=True, stop=True,
                                     tile_position=(r, 0))
                    e = epool.tile([128, RPC], BF, name=f"e{h}_{kc}", tag=f"e{h}")
                    nc.scalar.activation(out=e[:], in_=ps_s[:], func=AF.Exp, scale=SCALE)
                    es.append(e)
                for h in range(HEADS):
                    nc.tensor.matmul(
                        out=ctxp[h][:],
                        lhsT=vext[kc][:, h * (DH + 1):(h + 1) * (DH + 1)],
                        rhs=es[h][:], start=(kc == 0), stop=(kc == NCH - 1))

            ph5q.__exit__(None, None, None)
            ph5b = tc.tile_pool(name="bc_ps", bufs=1, space="PSUM")
            bc_ps = ph5b.__enter__()
            ctxT = [singles.tile([128, RPC], BF, name=f"ctxT{j}") for j in range(HCH)]
            ones1 = singles.tile([1, DH], F32)
            nc.vector.memset(ones1[:], 1.0)
            for h in range(HEADS):
                rs = singles.tile([1, RPC], F32, name=f"rs{h}", tag="rs")
                nc.vector.tensor_copy(out=rs[:], in_=ctxp[h][DH:DH + 1, :])
                rrec = singles.tile([1, RPC], F32, name=f"rrec{h}", tag="rrec")
                nc.vector.reciprocal(out=rrec[:], in_=rs[:])
                bc = bc_ps.tile([DH, RPC], F32, name=f"bc{h}", tag="bc")
                nc.tensor.matmul(out=bc[:], lhsT=ones1[:], rhs=rrec[:], start=True, stop=True)
                cs = singles.tile([DH, RPC], F32, name=f"cs{h}", tag="cs")
                nc.vector.tensor_copy(out=cs[:], in_=bc[:])
                j, r = h // 2, (h % 2) * 64
                nc.vector.tensor_tensor(out=ctxT[j][r:r + 64, :], in0=ctxp[h][0:DH, :],
                                        in1=cs[:], op=mybir.AluOpType.mult)

            ph5b.__exit__(None, None, None)
            ph5c.__exit__(None, None, None)
            ph6 = tc.tile_pool(name="fin_ps", bufs=2, space="PSUM")
            mid_ps = ph6.__enter__()

            # attn_out rows + residual + LN1
            xrows = [singles.tile([128, H], F32, name=f"xrows{i}") for i in range(NBLK)]
            for i in range(NBLK):
                ps = mid_ps.tile([128, H], F32)
                for k in range(HCH):
                    nc.tensor.matmul(out=ps[:], lhsT=ctxT[k][:, i * 128:(i + 1) * 128],
                                     rhs=Wo[k][:], start=(k == 0), stop=(k == HCH - 1))
                z = singles.tile([128, H], F32, name=f"z{i}", tag="zrow")
                nc.vector.tensor_add(z[:], ps[:], bob[:])
                nc.vector.tensor_add(z[:], z[:], hrows[i][:])
                _layernorm_rows(nc, singles, z, xrows[i], g1b, be1b, i, "ln1", epst)
            xT = [singles.tile([128, RPC], BF, name=f"xT{j}") for j in range(HCH)]
            for i in range(NBLK):
                for j in range(HCH):
                    tp = mid_ps.tile([128, 128], F32, name=f"tpx_{i}_{j}", tag="tp")
                    nc.tensor.transpose(out=tp[:], in_=xrows[i][:, j * 128:(j + 1) * 128],
                                        identity=ident[:])
                    nc.vector.tensor_copy(out=xT[j][:, i * 128:(i + 1) * 128], in_=tp[:])

            # FFN + LN2
            y1T = [singles.tile([128, RPC], BF, name=f"y1T{j}") for j in range(4)]
            for j in range(4):
                ps = mid_ps.tile([128, RPC], F32)
                for k in range(HCH):
                    nc.tensor.matmul(out=ps[:], lhsT=W1[k][:, j * 128:(j + 1) * 128],
                                     rhs=xT[k][:], start=(k == 0), stop=(k == HCH - 1))
                nc.scalar.activation(out=y1T[j][:], in_=ps[:], func=AF.Gelu,
                                     bias=b1c[:, j:j + 1], scale=1.0)
            for i in range(NBLK):
                ps = mid_ps.tile([128, H], F32)
                for k in range(4):
                    nc.tensor.matmul(out=ps[:], lhsT=y1T[k][:, i * 128:(i + 1) * 128],
                                     rhs=W2[k][:], start=(k == 0), stop=(k == 3))
                z = singles.tile([128, H], F32, name=f"z2{i}", tag="z2row")
                nc.vector.tensor_add(z[:], ps[:], b2b[:])
                nc.vector.tensor_add(z[:], z[:], xrows[i][:])
                o = singles.tile([128, H], F32, name=f"o{i}", tag="orow")
                _layernorm_rows(nc, singles, z, o, g2b, be2b, i, "ln2", epst)
                nc.sync.dma_start(out=out[i * 128:(i + 1) * 128, :], in_=o[:])
            ph6.__exit__(None, None, None)

    nc.compile()
    return nc


def _prep_edges(edge_index):
    src = np.asarray(edge_index[0]).astype(np.int64)
    dst = np.asarray(edge_index[1]).astype(np.int64)
    order = np.argsort(dst, kind="stable")
    src_s = src[order].astype(np.int32)
    dst_s = dst[order].astype(np.int32)
    blk = (dst_s // 128).astype(np.int64)
    counts = np.bincount(blk, minlength=N // 128)
    assert counts.max() <= TPB * 128, f"dst block overflow: {counts.max()} > {TPB*128}"
    starts = np.concatenate([[0], np.cumsum(counts)])
    per_core = []
    for c in range(NCORES):
        sT = np.full((NT * 128,), N, np.int32)
        dT = np.zeros((NT * 128,), np.int32)
        for b in range(NBLK):
            gb = c * NBLK + b
            s0, s1 = starts[gb], starts[gb + 1]
            n = s1 - s0
            o = b * TPB * 128
            sT[o:o + n] = src_s[s0:s1]
            dT[o:o + n] = dst_s[s0:s1] - c * RPC
            dT[o + n:o + TPB * 128] = b * 128
        per_core.append((np.ascontiguousarray(sT.reshape(NT, 128).T),
                         np.ascontiguousarray(dT.reshape(NT, 128).T)))
    return per_core


def kernel(**inputs):
    h = np.asarray(inputs["h"], np.float32)
    if "prog" not in _CACHE:
        _CACHE["prog"] = _build_program()
    nc = _CACHE["prog"]

    bf = ml_dtypes.bfloat16
    hT = np.ascontiguousarray(h.T).astype(bf)
    per_core_edges = _prep_edges(inputs["edge_index"])

    W = {k: np.asarray(inputs[k], np.float32) for k in
         ("W_src", "W_tgt", "Wq", "Wk", "Wv", "Wo", "W1", "W2")}
    B = {k: np.asarray(inputs[k], np.float32) for k in
         ("b_src", "b_tgt", "bq", "bk", "bv", "bo", "b1", "b2", "g1", "be1", "g2", "be2")}

    def bcast(v):
        return np.ascontiguousarray(np.tile(v[None, :], (128, 1)).astype(np.float32))

    common = {
        "hT_bf": hT,
        "w_src": W["W_src"].astype(bf), "w_tgt": W["W_tgt"].astype(bf),
        "w_q": W["Wq"].astype(bf), "w_k": W["Wk"].astype(bf),
        "w_v": W["Wv"].astype(bf), "w_o": W["Wo"].astype(bf),
        "w_1": W["W1"].astype(bf), "w_2": W["W2"].astype(bf),
        "bst_b": bcast(B["b_src"] + B["b_tgt"]),
        "bq_c": np.ascontiguousarray(B["bq"].reshape(HCH, 128).T.astype(np.float32)),
        "bk_c": np.ascontiguousarray(B["bk"].reshape(HCH, 128).T.astype(np.float32)),
        "bv_b": bcast(B["bv"]),
        "bo_b": bcast(B["bo"]),
        "b1_c": np.ascontiguousarray(B["b1"].reshape(4, 128).T.astype(np.float32)),
        "b2_b": bcast(B["b2"]),
        "g1_b": bcast(B["g1"]),
        "be1_b": bcast(B["be1"]),
        "g2_b": bcast(B["g2"]),
        "be2_b": bcast(B["be2"]),
    }
    in_maps = []
    for c in range(NCORES):
        sT, dT = per_core_edges[c]
        m = dict(common)
        m["srcT"] = sT
        m["dstT"] = dT
        m["h_rows"] = np.ascontiguousarray(h[c * RPC:(c + 1) * RPC, :])
        m["hT_own"] = np.ascontiguousarray(hT[:, c * RPC:(c + 1) * RPC])
        in_maps.append(m)

    res = run_bass_kernel_spmd(nc, in_maps, list(range(NCORES)))
    return np.concatenate([res.results[c]["out"] for c in range(NCORES)], axis=0)


if __name__ == "__main__":
    import reference
    inp = reference.setup_inputs()
    outp = kernel(**{k: np.asarray(v) for k, v in inp.items()})
    print("kernel out:", outp.shape, outp.dtype)



# revision 7
# speedup vs baseline: 2.6657x; 2.6657x over previous
"""DAGCN layer kernel for 8 Trainium2 NeuronCores (Bass/Tile, SPMD).

Dense reformulation of the edge MLP + segment_sum (equal to the reference
by linearity):
  Cn[s, d] = #edges(s->d) / max(deg(d), 1)   (host-built, bf16)
  m[d]     = 1 if deg(d) > 0 else 0
  aggT = Wsrc^T (h^T Cn) + Wtgt^T (hTm_own) + bst x m      (all on PE)
where hTm_own = hT_own * m and bst = b_src + b_tgt.
bk is dropped (softmax-invariant); bv is folded into bo (attn rows sum to 1).

Then: AllGather of aggT (bf16), kT/vext, multi-head attention with
software-pipelined exp, FFN with LayerNorms. Core c owns rows
[512c, 512c+512).
"""

import contextlib
import numpy as np
import ml_dtypes

import concourse.bass as bass
import concourse.bacc as bacc
import concourse.tile as tile
from concourse import mybir
from concourse.bass_utils import run_bass_kernel_spmd
from concourse.masks import make_identity

N, H, HEADS, E = 4096, 256, 4, 262144
DH = H // HEADS          # 64
NCORES = 8
RPC = N // NCORES        # 512 nodes per core
NBLK = RPC // 128        # 4 row blocks per core
NCH = N // 128           # 32
HCH = H // 128           # 2
EPS = 1e-5

BF = mybir.dt.bfloat16
F32 = mybir.dt.float32
AF = mybir.ActivationFunctionType

# wcat column offsets (per 128-row chunk): Wsrc Wtgt Wq Wk Wv Wo W1 W2a W2b
O_SRC, O_TGT, O_Q, O_K, O_V, O_O, O_1, O_2A, O_2B = (
    0, 256, 512, 768, 1024, 1280, 1536, 2048, 2304)
WCAT_COLS = 2560
# bcat column offsets: bo_eff b2 g1 be1 g2 be2 bq(2) b1(4)
B_O, B_2, B_G1, B_BE1, B_G2, B_BE2, B_Q, B_1 = (
    0, 256, 512, 768, 1024, 1280, 1536, 1538)
BCAT_COLS = 1542

_CACHE = {}


def _layernorm_rows(nc, pool, z, out_ap, g, be, i, nm, epst):
    """LayerNorm along the free dim of a [128, H] f32 tile; writes out_ap."""
    stats = pool.tile([128, 6], F32, name=f"{nm}_st{i}", tag=f"{nm}_st")
    nc.vector.bn_stats(out=stats[:], in_=z[:])
    mv = pool.tile([128, 2], F32, name=f"{nm}_mv{i}", tag=f"{nm}_mv")
    nc.vector.bn_aggr(out=mv[:], in_=stats[:])
    sd = pool.tile([128, 1], F32, name=f"{nm}_sd{i}", tag=f"{nm}_sd")
    nc.scalar.activation(out=sd[:], in_=mv[:, 1:2], func=AF.Sqrt,
                         bias=epst[:, 0:1], scale=1.0)
    rstd = pool.tile([128, 1], F32, name=f"{nm}_rs{i}", tag=f"{nm}_rs")
    nc.vector.reciprocal(out=rstd[:], in_=sd[:])
    nmu = pool.tile([128, 1], F32, name=f"{nm}_nm{i}", tag=f"{nm}_nm")
    nc.vector.tensor_tensor(out=nmu[:], in0=mv[:, 0:1], in1=rstd[:],
                            op=mybir.AluOpType.mult)
    nc.vector.tensor_scalar_mul(nmu[:], nmu[:], -1.0)
    zn = pool.tile([128, z.shape[1]], F32, name=f"{nm}_zn{i}", tag=f"{nm}_zn")
    nc.scalar.activation(out=zn[:], in_=z[:], func=AF.Identity,
                         bias=nmu[:, 0:1], scale=rstd[:, 0:1])
    nc.vector.tensor_tensor(out=zn[:], in0=zn[:], in1=g, op=mybir.AluOpType.mult)
    nc.vector.tensor_tensor(out_ap, in0=zn[:], in1=be, op=mybir.AluOpType.add)


def _build_program():
    nc = bacc.Bacc("TRN2", target_bir_lowering=False, debug=False,
                   num_devices=NCORES)

    h_bf_d = nc.dram_tensor("h_bf", [N, H], BF, kind="ExternalInput")
    cn_d = nc.dram_tensor("cn", [N, RPC], BF, kind="ExternalInput")
    hcat_d = nc.dram_tensor("hcat", [4 * 128, RPC], BF, kind="ExternalInput")
    hrows_d = nc.dram_tensor("h_rows", [RPC, H], F32, kind="ExternalInput")
    wcat_d = nc.dram_tensor("wcat", [H, WCAT_COLS], BF, kind="ExternalInput")
    bcat_d = nc.dram_tensor("bcat", [128, BCAT_COLS], F32, kind="ExternalInput")
    sm_d = nc.dram_tensor("sm1", [1, H + RPC], BF, kind="ExternalInput")
    out_d = nc.dram_tensor("out", [RPC, H], F32, kind="ExternalOutput")

    with tile.TileContext(nc) as tc, contextlib.ExitStack() as ctx:
        singles = ctx.enter_context(tc.tile_pool(name="singles", bufs=1))
        epool = ctx.enter_context(tc.tile_pool(name="epool", bufs=2))
        big = tc.tile_pool(name="big", bufs=1)
        bigp = big.__enter__()

        # ---------- input DMAs (batched) ----------
        cnb = bigp.tile([128, NCH * RPC], BF, name="cnb")
        nc.sync.dma_start(
            out=cnb[:].rearrange("p (s f) -> p s f", f=RPC),
            in_=cn_d[:].rearrange("(s p) f -> p s f", p=128))
        hbf = bigp.tile([128, NCH * H], BF, name="hbf")
        nc.sync.dma_start(
            out=hbf[:].rearrange("p (s f) -> p s f", f=H),
            in_=h_bf_d[:].rearrange("(s p) f -> p s f", p=128))
        hc = singles.tile([128, 4 * RPC], BF, name="hc")
        nc.sync.dma_start(
            out=hc[:].rearrange("p (c f) -> p c f", f=RPC),
            in_=hcat_d[:].rearrange("(c p) f -> p c f", p=128))
        hr = singles.tile([128, NBLK * H], F32, name="hr")
        nc.sync.dma_start(
            out=hr[:].rearrange("p (b f) -> p b f", f=H),
            in_=hrows_d[:].rearrange("(b p) f -> p b f", p=128))
        wct = [singles.tile([128, WCAT_COLS], BF, name=f"wct{k}")
               for k in range(HCH)]
        for k in range(HCH):
            nc.sync.dma_start(out=wct[k][:], in_=wcat_d[k * 128:(k + 1) * 128, :])
        bct = singles.tile([128, BCAT_COLS], F32, name="bct")
        nc.sync.dma_start(out=bct[:], in_=bcat_d[:])
        sm = singles.tile([1, H + RPC], BF, name="sm")
        nc.sync.dma_start(out=sm[:], in_=sm_d[:])

        ident = singles.tile([128, 128], F32)
        make_identity(nc, ident[:])
        epst = singles.tile([128, 1], F32)
        nc.vector.memset(epst[:], EPS)

        # ---------- P1: U = h^T Cn ; aggT ; qT ----------
        pre = tc.tile_pool(name="pre_ps", bufs=1, space="PSUM")
        pre_ps = pre.__enter__()
        U = [pre_ps.tile([128, RPC], F32, name=f"U{j}") for j in range(HCH)]
        for s in range(NCH):
            for j in range(HCH):
                nc.tensor.matmul(
                    out=U[j][:],
                    lhsT=hbf[:, s * H + j * 128: s * H + (j + 1) * 128],
                    rhs=cnb[:, s * RPC:(s + 1) * RPC],
                    start=(s == 0), stop=(s == NCH - 1))
        u_sb = [singles.tile([128, RPC], BF, name=f"u_sb{j}") for j in range(HCH)]
        for j in range(HCH):
            nc.scalar.copy(out=u_sb[j][:], in_=U[j][:])

        aggTo = [singles.tile([128, RPC], BF, name=f"aggTo{j}") for j in range(HCH)]
        for j in range(HCH):
            ag = pre_ps.tile([128, RPC], F32, name=f"agg{j}")
            nc.tensor.matmul(out=ag[:], lhsT=wct[0][:, O_SRC + j * 128:O_SRC + (j + 1) * 128],
                             rhs=u_sb[0][:], start=True, stop=False)
            nc.tensor.matmul(out=ag[:], lhsT=wct[1][:, O_SRC + j * 128:O_SRC + (j + 1) * 128],
                             rhs=u_sb[1][:], start=False, stop=False)
            nc.tensor.matmul(out=ag[:], lhsT=wct[0][:, O_TGT + j * 128:O_TGT + (j + 1) * 128],
                             rhs=hc[:, 2 * RPC:3 * RPC], start=False, stop=False)
            nc.tensor.matmul(out=ag[:], lhsT=wct[1][:, O_TGT + j * 128:O_TGT + (j + 1) * 128],
                             rhs=hc[:, 3 * RPC:4 * RPC], start=False, stop=False)
            nc.tensor.matmul(out=ag[:], lhsT=sm[0:1, j * 128:(j + 1) * 128],
                             rhs=sm[0:1, H:H + RPC], start=False, stop=True)
            nc.vector.tensor_copy(out=aggTo[j][:], in_=ag[:])

        qT = [singles.tile([128, RPC], BF, name=f"qT{j}") for j in range(HCH)]
        for j in range(HCH):
            qp = pre_ps.tile([128, RPC], F32, name=f"qp{j}")
            for k in range(HCH):
                nc.tensor.matmul(out=qp[:], lhsT=wct[k][:, O_Q + j * 128:O_Q + (j + 1) * 128],
                                 rhs=hc[:, k * RPC:(k + 1) * RPC],
                                 start=(k == 0), stop=(k == HCH - 1))
            nc.scalar.activation(out=qT[j][:], in_=qp[:], func=AF.Identity,
                                 bias=bct[:, B_Q + j:B_Q + j + 1], scale=1.0)
        pre.__exit__(None, None, None)

        # ---------- P2: AllGather aggT ----------
        with tc.tile_pool(name="dram", bufs=1, space="DRAM") as dram:
            cc_in = dram.tile([H, RPC], BF)
            cc_out = dram.tile([NCORES * H, RPC], BF, addr_space="Shared")
            for j in range(HCH):
                nc.gpsimd.dma_start(out=cc_in[j * 128:(j + 1) * 128, :],
                                    in_=aggTo[j][:])
            nc.gpsimd.collective_compute(
                "AllGather", mybir.AluOpType.bypass,
                replica_groups=[list(range(NCORES))],
                ins=[cc_in.opt()], outs=[cc_out.opt()])
            aggT_full = [singles.tile([128, N], BF, name=f"aggTf{j}")
                         for j in range(HCH)]
            ccv = cc_out[:].rearrange("(c j p) f -> j p c f", c=NCORES, p=128)
            for j in range(HCH):
                nc.sync.dma_start(
                    out=aggT_full[j][:].rearrange("p (c f) -> p c f", f=RPC),
                    in_=ccv[j])

            # ---------- P3: kT and vext ----------
            kv = tc.tile_pool(name="kv_ps", bufs=2, space="PSUM")
            kv_ps = kv.__enter__()
            kT = [singles.tile([128, N], BF, name=f"kT{j}") for j in range(HCH)]
            for j in range(HCH):
                for piece in range(N // RPC):
                    ps = kv_ps.tile([128, RPC], F32, tag="kps")
                    for k in range(HCH):
                        nc.tensor.matmul(
                            out=ps[:], lhsT=wct[k][:, O_K + j * 128:O_K + (j + 1) * 128],
                            rhs=aggT_full[k][:, piece * RPC:(piece + 1) * RPC],
                            start=(k == 0), stop=(k == HCH - 1))
                    if j == 0:
                        nc.scalar.copy(out=kT[j][:, piece * RPC:(piece + 1) * RPC],
                                       in_=ps[:])
                    else:
                        nc.vector.tensor_copy(out=kT[j][:, piece * RPC:(piece + 1) * RPC],
                                              in_=ps[:])
            vext = [singles.tile([128, HEADS * (DH + 1)], BF, name=f"vext{kc}")
                    for kc in range(NCH)]
            for kc in range(NCH):
                vp = kv_ps.tile([128, H], F32, tag="vps")
                for k in range(HCH):
                    nc.tensor.matmul(out=vp[:],
                                     lhsT=aggT_full[k][:, kc * 128:(kc + 1) * 128],
                                     rhs=wct[k][:, O_V:O_V + H],
                                     start=(k == 0), stop=(k == HCH - 1))
                vv = vext[kc][:].rearrange("p (h d) -> p h d", d=DH + 1)
                nc.vector.memset(vv[:, :, DH:DH + 1], 1.0)
                nc.vector.tensor_copy(
                    out=vv[:, :, 0:DH],
                    in_=vp[:].rearrange("p (h d) -> p h d", d=DH))
            kv.__exit__(None, None, None)

            # ---------- P4: attention (pipelined exp) ----------
            SCALE = float(1.0 / np.sqrt(DH))
            ctxpool = tc.tile_pool(name="ctx_ps", bufs=1, space="PSUM")
            ctx_ps = ctxpool.__enter__()
            qk = tc.tile_pool(name="qk_ps", bufs=2, space="PSUM")
            qk_ps = qk.__enter__()
            ctxp = [ctx_ps.tile([DH + 1, RPC], F32, name=f"ctxp{h}")
                    for h in range(HEADS)]

            def scores(kc):
                es_pair = []
                for pair in range(HCH):
                    sc = qk_ps.tile([128, 2 * RPC], F32, tag="sc",
                                    name=f"sc{pair}_{kc}")
                    for r in (0, 64):
                        nc.tensor.matmul(
                            out=sc[:, (r // 64) * RPC:(r // 64 + 1) * RPC],
                            lhsT=kT[pair][r:r + 64, kc * 128:(kc + 1) * 128],
                            rhs=qT[pair][r:r + 64, :], start=True, stop=True,
                            tile_position=(r, 0))
                    e = epool.tile([128, 2 * RPC], BF, tag=f"es{pair}",
                                   name=f"es{pair}_{kc}")
                    nc.scalar.activation(out=e[:], in_=sc[:], func=AF.Exp,
                                         scale=SCALE)
                    es_pair.append(e)
                return es_pair

            def ctx_mm(kc, es_pair):
                for h in range(HEADS):
                    nc.tensor.matmul(
                        out=ctxp[h][:],
                        lhsT=vext[kc][:, h * (DH + 1):(h + 1) * (DH + 1)],
                        rhs=es_pair[h // 2][:, (h % 2) * RPC:(h % 2 + 1) * RPC],
                        start=(kc == 0), stop=(kc == NCH - 1))

            prev = scores(0)
            for kc in range(1, NCH):
                cur = scores(kc)
                ctx_mm(kc - 1, prev)
                prev = cur
            ctx_mm(NCH - 1, prev)
            qk.__exit__(None, None, None)

            # softmax normalize -> ctxT (bf16)
            bcp = tc.tile_pool(name="bc_ps", bufs=1, space="PSUM")
            bc_ps = bcp.__enter__()
            ctxT = [singles.tile([128, RPC], BF, name=f"ctxT{j}") for j in range(HCH)]
            ones1 = singles.tile([1, DH], F32)
            nc.vector.memset(ones1[:], 1.0)
            for h in range(HEADS):
                rs = singles.tile([1, RPC], F32, name=f"rs{h}", tag="rs")
                nc.vector.tensor_copy(out=rs[:], in_=ctxp[h][DH:DH + 1, :])
                rrec = singles.tile([1, RPC], F32, name=f"rrec{h}", tag="rrec")
                nc.vector.reciprocal(out=rrec[:], in_=rs[:])
                bc = bc_ps.tile([DH, RPC], F32, name=f"bc{h}", tag="bc")
                nc.tensor.matmul(out=bc[:], lhsT=ones1[:], rhs=rrec[:],
                                 start=True, stop=True)
                cs = singles.tile([DH, RPC], F32, name=f"cs{h}", tag="cs")
                nc.vector.tensor_copy(out=cs[:], in_=bc[:])
                j, r = h // 2, (h % 2) * 64
                nc.vector.tensor_tensor(out=ctxT[j][r:r + 64, :],
                                        in0=ctxp[h][0:DH, :], in1=cs[:],
                                        op=mybir.AluOpType.mult)
            bcp.__exit__(None, None, None)
            ctxpool.__exit__(None, None, None)

            # ---------- P5: attn_out + LN1, FFN + LN2, store ----------
            fin = tc.tile_pool(name="fin_ps", bufs=2, space="PSUM")
            fin_ps = fin.__enter__()
            xrows = [singles.tile([128, H], F32, name=f"xrows{i}")
                     for i in range(NBLK)]
            for i in range(NBLK):
                ps = fin_ps.tile([128, H], F32, tag="ops")
                for k in range(HCH):
                    nc.tensor.matmul(out=ps[:], lhsT=ctxT[k][:, i * 128:(i + 1) * 128],
                                     rhs=wct[k][:, O_O:O_O + H],
                                     start=(k == 0), stop=(k == HCH - 1))
                z = singles.tile([128, H], F32, name=f"z{i}", tag="zrow")
                nc.vector.tensor_tensor(z[:], in0=ps[:], in1=bct[:, B_O:B_O + H],
                                        op=mybir.AluOpType.add)
                nc.vector.tensor_tensor(z[:], in0=z[:], in1=hr[:, i * H:(i + 1) * H],
                                        op=mybir.AluOpType.add)
                _layernorm_rows(nc, singles, z, xrows[i][:],
                                bct[:, B_G1:B_G1 + H], bct[:, B_BE1:B_BE1 + H],
                                i, "ln1", epst)
            xT = [singles.tile([128, RPC], BF, name=f"xT{j}") for j in range(HCH)]
            for i in range(NBLK):
                for j in range(HCH):
                    tp = fin_ps.tile([128, 128], F32, tag="tp")
                    nc.tensor.transpose(out=tp[:], in_=xrows[i][:, j * 128:(j + 1) * 128],
                                        identity=ident[:])
                    nc.vector.tensor_copy(out=xT[j][:, i * 128:(i + 1) * 128], in_=tp[:])

            y1T = [singles.tile([128, RPC], BF, name=f"y1T{j}") for j in range(4)]
            for j in range(4):
                ps = fin_ps.tile([128, RPC], F32, tag="y1ps")
                for k in range(HCH):
                    nc.tensor.matmul(out=ps[:], lhsT=wct[k][:, O_1 + j * 128:O_1 + (j + 1) * 128],
                                     rhs=xT[k][:], start=(k == 0), stop=(k == HCH - 1))
                nc.scalar.activation(out=y1T[j][:], in_=ps[:], func=AF.Gelu,
                                     bias=bct[:, B_1 + j:B_1 + j + 1], scale=1.0)

            W2CH = [(0, O_2A), (1, O_2A), (0, O_2B), (1, O_2B)]
            o_all = singles.tile([128, NBLK * H], F32, name="o_all")
            for i in range(NBLK):
                ps = fin_ps.tile([128, H], F32, tag="o2ps")
                for k2 in range(4):
                    kk, oo = W2CH[k2]
                    nc.tensor.matmul(out=ps[:], lhsT=y1T[k2][:, i * 128:(i + 1) * 128],
                                     rhs=wct[kk][:, oo:oo + H],
                                     start=(k2 == 0), stop=(k2 == 3))
                z = singles.tile([128, H], F32, name=f"z2{i}", tag="z2row")
                nc.vector.tensor_tensor(z[:], in0=ps[:], in1=bct[:, B_2:B_2 + H],
                                        op=mybir.AluOpType.add)
                nc.vector.tensor_tensor(z[:], in0=z[:], in1=xrows[i][:],
                                        op=mybir.AluOpType.add)
                _layernorm_rows(nc, singles, z, o_all[:, i * H:(i + 1) * H],
                                bct[:, B_G2:B_G2 + H], bct[:, B_BE2:B_BE2 + H],
                                i, "ln2", epst)
            fin.__exit__(None, None, None)
            nc.sync.dma_start(
                out=out_d[:].rearrange("(b p) f -> p b f", p=128),
                in_=o_all[:].rearrange("p (b f) -> p b f", f=H))
        big.__exit__(None, None, None)

    nc.compile()
    return nc


def _prep_graph(edge_index):
    src = np.asarray(edge_index[0]).astype(np.int64)
    dst = np.asarray(edge_index[1]).astype(np.int64)
    counts = np.bincount(src * N + dst, minlength=N * N).astype(np.float32)
    counts = counts.reshape(N, N)
    deg = counts.sum(axis=0)
    cn = counts / np.maximum(deg, 1.0)[None, :]
    m = (deg > 0).astype(np.float32)
    return cn, m


def _build_in_maps(inputs):
    bf = ml_dtypes.bfloat16
    h = np.asarray(inputs["h"], np.float32)
    cn, m = _prep_graph(inputs["edge_index"])
    W = {k: np.asarray(inputs[k], np.float32) for k in
         ("W_src", "W_tgt", "Wq", "Wk", "Wv", "Wo", "W1", "W2")}
    B = {k: np.asarray(inputs[k], np.float32) for k in
         ("b_src", "b_tgt", "bq", "bk", "bv", "bo", "b1", "b2",
          "g1", "be1", "g2", "be2")}

    wcat = np.concatenate(
        [W["W_src"], W["W_tgt"], W["Wq"], W["Wk"], W["Wv"], W["Wo"],
         W["W1"], W["W2"][0:256, :], W["W2"][256:512, :]], axis=1)
    assert wcat.shape == (H, WCAT_COLS)

    bo_eff = B["bo"] + B["bv"] @ W["Wo"]

    def bcast(v):
        return np.tile(v[None, :], (128, 1)).astype(np.float32)

    bcat = np.concatenate(
        [bcast(bo_eff), bcast(B["b2"]), bcast(B["g1"]), bcast(B["be1"]),
         bcast(B["g2"]), bcast(B["be2"]),
         np.ascontiguousarray(B["bq"].reshape(HCH, 128).T.astype(np.float32)),
         np.ascontiguousarray(B["b1"].reshape(4, 128).T.astype(np.float32))],
        axis=1)
    assert bcat.shape == (128, BCAT_COLS)

    bst = (B["b_src"] + B["b_tgt"]).astype(np.float32)
    hT = np.ascontiguousarray(h.T)

    common = {
        "h_bf": h.astype(bf),
        "wcat": np.ascontiguousarray(wcat).astype(bf),
        "bcat": np.ascontiguousarray(bcat),
    }
    in_maps = []
    for c in range(NCORES):
        sl = slice(c * RPC, (c + 1) * RPC)
        mc = m[sl]
        hTo = hT[:, sl]
        hcat = np.concatenate([hTo, hTo * mc[None, :]], axis=0)
        mm_ = dict(common)
        mm_["cn"] = np.ascontiguousarray(cn[:, sl]).astype(bf)
        mm_["hcat"] = np.ascontiguousarray(hcat).astype(bf)
        mm_["h_rows"] = np.ascontiguousarray(h[sl, :])
        mm_["sm1"] = np.ascontiguousarray(
            np.concatenate([bst, mc])[None, :]).astype(bf)
        in_maps.append(mm_)
    return in_maps


def kernel(**inputs):
    if "prog" not in _CACHE:
        _CACHE["prog"] = _build_program()
    nc = _CACHE["prog"]
    in_maps = _build_in_maps(inputs)
    res = run_bass_kernel_spmd(nc, in_maps, list(range(NCORES)))
    return np.concatenate([res.results[c]["out"] for c in range(NCORES)], axis=0)


if __name__ == "__main__":
    import reference
    inp = reference.setup_inputs()
    outp = kernel(**{k: np.asarray(v) for k, v in inp.items()})
    print("kernel out:", outp.shape, outp.dtype)


# revision 20
# speedup vs baseline: 2.8310x; 1.0620x over previous
"""DAGCN layer kernel for 8 Trainium2 NeuronCores (Bass/Tile, SPMD).

Dense reformulation of the edge MLP + segment_sum (equal to the reference
by linearity):
  Cn[s, d] = #edges(s->d) / max(deg(d), 1)   (host-built, bf16)
  m[d]     = 1 if deg(d) > 0 else 0
  aggT = Wsrc^T (h^T Cn) + Wtgt^T (hTm_own) + bst x m      (all on PE)
where hTm_own = hT_own * m and bst = b_src + b_tgt.
bk is dropped (softmax-invariant); bv is folded into bo (attn rows sum to 1).

Then: AllGather of aggT (bf16), kT/vext, multi-head attention with
software-pipelined exp, FFN with LayerNorms. Core c owns rows
[512c, 512c+512).
"""

import contextlib
import numpy as np
import ml_dtypes

import concourse.bass as bass
import concourse.bacc as bacc
import concourse.tile as tile
from concourse import mybir
from concourse.bass_utils import run_bass_kernel_spmd
from concourse.masks import make_identity

N, H, HEADS, E = 4096, 256, 4, 262144
DH = H // HEADS          # 64
NCORES = 8
RPC = N // NCORES        # 512 nodes per core
NBLK = RPC // 128        # 4 row blocks per core
NCH = N // 128           # 32
HCH = H // 128           # 2
EPS = 1e-5

BF = mybir.dt.bfloat16
F32 = mybir.dt.float32
AF = mybir.ActivationFunctionType

# wcat column offsets (per 128-row chunk): Wsrc Wtgt Wq Wk Wv Wo W1 W2a W2b
O_SRC, O_TGT, O_Q, O_K, O_V, O_O, O_1, O_2A, O_2B = (
    0, 256, 512, 768, 1024, 1280, 1536, 2048, 2304)
WCAT_COLS = 2560
# bcat column offsets: bo_eff b2 g1 be1 g2 be2 bq(2) b1(4)
B_O, B_2, B_G1, B_BE1, B_G2, B_BE2, B_Q, B_1 = (
    0, 256, 512, 768, 1024, 1280, 1536, 1538)
BCAT_COLS = 1542

_CACHE = {}


def _layernorm_rows(nc, pool, z, out_ap, g, be, i, nm, epst):
    """LayerNorm along the free dim of a [128, H] f32 tile; writes out_ap."""
    stats = pool.tile([128, 6], F32, name=f"{nm}_st{i}")
    nc.vector.bn_stats(out=stats[:], in_=z[:])
    mv = pool.tile([128, 2], F32, name=f"{nm}_mv{i}")
    nc.vector.bn_aggr(out=mv[:], in_=stats[:])
    sd = pool.tile([128, 1], F32, name=f"{nm}_sd{i}")
    nc.scalar.activation(out=sd[:], in_=mv[:, 1:2], func=AF.Sqrt,
                         bias=epst[:, 0:1], scale=1.0)
    rstd = pool.tile([128, 1], F32, name=f"{nm}_rs{i}")
    nc.vector.reciprocal(out=rstd[:], in_=sd[:])
    nmu = pool.tile([128, 1], F32, name=f"{nm}_nm{i}")
    nc.vector.tensor_tensor(out=nmu[:], in0=mv[:, 0:1], in1=rstd[:],
                            op=mybir.AluOpType.mult)
    nc.vector.tensor_scalar_mul(nmu[:], nmu[:], -1.0)
    zn = pool.tile([128, z.shape[1]], F32, name=f"{nm}_zn{i}")
    nc.scalar.activation(out=zn[:], in_=z[:], func=AF.Identity,
                         bias=nmu[:, 0:1], scale=rstd[:, 0:1])
    nc.vector.tensor_tensor(out=zn[:], in0=zn[:], in1=g, op=mybir.AluOpType.mult)
    nc.vector.tensor_tensor(out_ap, in0=zn[:], in1=be, op=mybir.AluOpType.add)


def _build_program():
    nc = bacc.Bacc("TRN2", target_bir_lowering=False, debug=False,
                   num_devices=NCORES)

    h_bf_d = nc.dram_tensor("h_bf", [N, H], BF, kind="ExternalInput")
    cn_d = nc.dram_tensor("cn", [N, RPC], BF, kind="ExternalInput")
    hcat_d = nc.dram_tensor("hcat", [4 * 128, RPC], BF, kind="ExternalInput")
    hrows_d = nc.dram_tensor("h_rows", [RPC, H], F32, kind="ExternalInput")
    wcat_d = nc.dram_tensor("wcat", [H, WCAT_COLS], BF, kind="ExternalInput")
    bcat_d = nc.dram_tensor("bcat", [128, BCAT_COLS], F32, kind="ExternalInput")
    sm_d = nc.dram_tensor("sm1", [1, H + RPC], BF, kind="ExternalInput")
    out_d = nc.dram_tensor("out", [RPC, H], F32, kind="ExternalOutput")

    with tile.TileContext(nc) as tc, contextlib.ExitStack() as ctx:
        singles = ctx.enter_context(tc.tile_pool(name="singles", bufs=1))
        epool = ctx.enter_context(tc.tile_pool(name="epool", bufs=2))
        big = tc.tile_pool(name="big", bufs=1)
        bigp = big.__enter__()

        # ---------- input DMAs (small first, big ones split for pipelining) ----------
        hc = singles.tile([128, 4 * RPC], BF, name="hc")
        nc.sync.dma_start(
            out=hc[:].rearrange("p (c f) -> p c f", f=RPC),
            in_=hcat_d[:].rearrange("(c p) f -> p c f", p=128))
        wct = [singles.tile([128, WCAT_COLS], BF, name=f"wct{k}")
               for k in range(HCH)]
        for k in range(HCH):
            nc.sync.dma_start(out=wct[k][:], in_=wcat_d[k * 128:(k + 1) * 128, :])
        bct = singles.tile([128, BCAT_COLS], F32, name="bct")
        nc.sync.dma_start(out=bct[:], in_=bcat_d[:])
        sm = singles.tile([1, H + RPC], BF, name="sm")
        nc.sync.dma_start(out=sm[:], in_=sm_d[:])
        hr = singles.tile([128, NBLK * H], F32, name="hr")
        nc.sync.dma_start(
            out=hr[:].rearrange("p (b f) -> p b f", f=H),
            in_=hrows_d[:].rearrange("(b p) f -> p b f", p=128))

        cnb = bigp.tile([128, NCH * RPC], BF, name="cnb")
        hbf = bigp.tile([128, NCH * H], BF, name="hbf")
        NPC = NCH // 8  # src chunks per DMA piece
        for pc in range(8):
            s0 = pc * NPC
            nc.sync.dma_start(
                out=cnb[:, s0 * RPC:(s0 + NPC) * RPC].rearrange(
                    "p (s f) -> p s f", f=RPC),
                in_=cn_d[s0 * 128:(s0 + NPC) * 128, :].rearrange(
                    "(s p) f -> p s f", p=128))
            nc.sync.dma_start(
                out=hbf[:, s0 * H:(s0 + NPC) * H].rearrange(
                    "p (s f) -> p s f", f=H),
                in_=h_bf_d[s0 * 128:(s0 + NPC) * 128, :].rearrange(
                    "(s p) f -> p s f", p=128))

        ident = singles.tile([128, 128], F32)
        make_identity(nc, ident[:])
        epst = singles.tile([128, 1], F32)
        nc.vector.memset(epst[:], EPS)

        # ---------- P1: U = h^T Cn ; aggT ; qT ; own k/v ----------
        pre = tc.tile_pool(name="pre_ps", bufs=2, space="PSUM")
        pre_ps = pre.__enter__()
        U = [pre_ps.tile([128, RPC], F32, tag="acc", name=f"U{j}")
             for j in range(HCH)]
        for s in range(NCH):
            for j in range(HCH):
                nc.tensor.matmul(
                    out=U[j][:],
                    lhsT=hbf[:, s * H + j * 128: s * H + (j + 1) * 128],
                    rhs=cnb[:, s * RPC:(s + 1) * RPC],
                    start=(s == 0), stop=(s == NCH - 1))
        u_sb = [singles.tile([128, RPC], BF, name=f"u_sb{j}") for j in range(HCH)]
        for j in range(HCH):
            nc.scalar.copy(out=u_sb[j][:], in_=U[j][:])

        aggTo = [singles.tile([128, RPC], BF, name=f"aggTo{j}") for j in range(HCH)]
        for j in range(HCH):
            ag = pre_ps.tile([128, RPC], F32, tag="acc", name=f"agg{j}")
            nc.tensor.matmul(out=ag[:], lhsT=wct[0][:, O_SRC + j * 128:O_SRC + (j + 1) * 128],
                             rhs=u_sb[0][:], start=True, stop=False)
            nc.tensor.matmul(out=ag[:], lhsT=wct[1][:, O_SRC + j * 128:O_SRC + (j + 1) * 128],
                             rhs=u_sb[1][:], start=False, stop=False)
            nc.tensor.matmul(out=ag[:], lhsT=wct[0][:, O_TGT + j * 128:O_TGT + (j + 1) * 128],
                             rhs=hc[:, 2 * RPC:3 * RPC], start=False, stop=False)
            nc.tensor.matmul(out=ag[:], lhsT=wct[1][:, O_TGT + j * 128:O_TGT + (j + 1) * 128],
                             rhs=hc[:, 3 * RPC:4 * RPC], start=False, stop=False)
            nc.tensor.matmul(out=ag[:], lhsT=sm[0:1, j * 128:(j + 1) * 128],
                             rhs=sm[0:1, H:H + RPC], start=False, stop=True)
            nc.vector.tensor_copy(out=aggTo[j][:], in_=ag[:])

        qT = [singles.tile([128, RPC], BF, name=f"qT{j}") for j in range(HCH)]
        for j in range(HCH):
            qp = pre_ps.tile([128, RPC], F32, tag="acc", name=f"qp{j}")
            for k in range(HCH):
                nc.tensor.matmul(out=qp[:], lhsT=wct[k][:, O_Q + j * 128:O_Q + (j + 1) * 128],
                                 rhs=hc[:, k * RPC:(k + 1) * RPC],
                                 start=(k == 0), stop=(k == HCH - 1))
            nc.scalar.activation(out=qT[j][:], in_=qp[:], func=AF.Identity,
                                 bias=bct[:, B_Q + j:B_Q + j + 1], scale=1.0)

        # own k slice: kTo = Wk^T aggT_own  [H, RPC]
        kTo = [singles.tile([128, RPC], BF, name=f"kTo{j}") for j in range(HCH)]
        for j in range(HCH):
            kp = pre_ps.tile([128, RPC], F32, tag="acc", name=f"kp{j}")
            for k in range(HCH):
                nc.tensor.matmul(out=kp[:], lhsT=wct[k][:, O_K + j * 128:O_K + (j + 1) * 128],
                                 rhs=aggTo[k][:], start=(k == 0), stop=(k == HCH - 1))
            nc.scalar.copy(out=kTo[j][:], in_=kp[:])
        # own v slice, extended with ones col per head: [128, NBLK*260]
        VW = HEADS * (DH + 1)  # 260
        vexto = singles.tile([128, NBLK * VW], BF, name="vexto")
        for kl in range(NBLK):
            vp = pre_ps.tile([128, H], F32, tag="acc2", name=f"vo{kl}")
            for k in range(HCH):
                nc.tensor.matmul(out=vp[:],
                                 lhsT=aggTo[k][:, kl * 128:(kl + 1) * 128],
                                 rhs=wct[k][:, O_V:O_V + H],
                                 start=(k == 0), stop=(k == HCH - 1))
            vv = vexto[:, kl * VW:(kl + 1) * VW].rearrange(
                "p (h d) -> p h d", d=DH + 1)
            nc.vector.memset(vv[:, :, DH:DH + 1], 1.0)
            nc.vector.tensor_copy(
                out=vv[:, :, 0:DH],
                in_=vp[:].rearrange("p (h d) -> p h d", d=DH))
        pre.__exit__(None, None, None)

        # ---------- P2: AllGather (kTo | vexto), unpack ----------
        KSZ = H * RPC                 # 131072 elems
        VSZ = RPC * VW                # 133120 elems
        CCW = KSZ + VSZ
        with tc.tile_pool(name="dram", bufs=1, space="DRAM") as dram:
            cc_in = dram.tile([CCW], BF)
            cc_out = dram.tile([NCORES * CCW], BF, addr_space="Shared")
            for j in range(HCH):
                nc.gpsimd.dma_start(
                    out=cc_in[j * 128 * RPC:(j + 1) * 128 * RPC].rearrange(
                        "(p f) -> p f", p=128),
                    in_=kTo[j][:])
            nc.gpsimd.dma_start(
                out=cc_in[KSZ:KSZ + VSZ].rearrange("(kl p f) -> p kl f", kl=NBLK, p=128),
                in_=vexto[:].rearrange("p (kl f) -> p kl f", f=VW))
            nc.gpsimd.collective_compute(
                "AllGather", mybir.AluOpType.bypass,
                replica_groups=[list(range(NCORES))],
                ins=[cc_in.opt()], outs=[cc_out.opt()])
            ccv = cc_out[:].rearrange("(c x) -> c x", c=NCORES)
            kT = [singles.tile([128, N], BF, name=f"kT{j}") for j in range(HCH)]
            for j in range(HCH):
                nc.sync.dma_start(
                    out=kT[j][:].rearrange("p (c f) -> p c f", f=RPC),
                    in_=ccv[:, j * 128 * RPC:(j + 1) * 128 * RPC].rearrange(
                        "c (p f) -> p c f", p=128))
            vext_big = bigp.tile([128, NCH * VW], BF, name="vext_big")
            for c in range(NCORES):
                nc.sync.dma_start(
                    out=vext_big[:, c * NBLK * VW:(c + 1) * NBLK * VW].rearrange(
                        "p (kl f) -> p kl f", f=VW),
                    in_=ccv[c, KSZ:KSZ + VSZ].rearrange(
                        "(kl p f) -> p kl f", kl=NBLK, p=128))

            def vext(kc):
                return vext_big[:, kc * VW:(kc + 1) * VW]

            # ---------- P4: attention (pipelined exp) ----------
            SCALE = float(1.0 / np.sqrt(DH))
            ctxpool = tc.tile_pool(name="ctx_ps", bufs=1, space="PSUM")
            ctx_ps = ctxpool.__enter__()
            qk = tc.tile_pool(name="qk_ps", bufs=2, space="PSUM")
            qk_ps = qk.__enter__()
            ctxp = [ctx_ps.tile([DH + 1, RPC], F32, name=f"ctxp{h}")
                    for h in range(HEADS)]

            def scores(kc):
                es_pair = []
                for pair in range(HCH):
                    sc = qk_ps.tile([128, 2 * RPC], F32, tag="sc",
                                    name=f"sc{pair}_{kc}")
                    for r in (0, 64):
                        nc.tensor.matmul(
                            out=sc[:, (r // 64) * RPC:(r // 64 + 1) * RPC],
                            lhsT=kT[pair][r:r + 64, kc * 128:(kc + 1) * 128],
                            rhs=qT[pair][r:r + 64, :], start=True, stop=True,
                            tile_position=(r, 0))
                    e = epool.tile([128, 2 * RPC], BF, tag=f"es{pair}",
                                   name=f"es{pair}_{kc}")
                    nc.scalar.activation(out=e[:], in_=sc[:], func=AF.Exp,
                                         scale=SCALE)
                    es_pair.append(e)
                return es_pair

            def ctx_mm(kc, es_pair):
                for h in range(HEADS):
                    nc.tensor.matmul(
                        out=ctxp[h][:],
                        lhsT=vext_big[:, kc * VW + h * (DH + 1):
                                      kc * VW + (h + 1) * (DH + 1)],
                        rhs=es_pair[h // 2][:, (h % 2) * RPC:(h % 2 + 1) * RPC],
                        start=(kc == 0), stop=(kc == NCH - 1))

            prev = scores(0)
            for kc in range(1, NCH):
                cur = scores(kc)
                ctx_mm(kc - 1, prev)
                prev = cur
            ctx_mm(NCH - 1, prev)
            qk.__exit__(None, None, None)

            # softmax normalize -> ctxT (bf16)
            ctxT = [singles.tile([128, RPC], BF, name=f"ctxT{j}") for j in range(HCH)]
            for h in range(HEADS):
                rs = singles.tile([1, RPC], F32, name=f"rs{h}")
                nc.scalar.copy(out=rs[:], in_=ctxp[h][DH:DH + 1, :])
                rrec = singles.tile([1, RPC], F32, name=f"rrec{h}")
                nc.vector.reciprocal(out=rrec[:], in_=rs[:])
                cs = singles.tile([DH, RPC], F32, name=f"cs{h}")
                nc.gpsimd.partition_broadcast(cs[:], rrec[:], channels=DH)
                j, r = h // 2, (h % 2) * 64
                nc.vector.tensor_tensor(out=ctxT[j][r:r + 64, :],
                                        in0=ctxp[h][0:DH, :], in1=cs[:],
                                        op=mybir.AluOpType.mult)
            ctxpool.__exit__(None, None, None)

            # ---------- P5: attn_out + LN1, FFN + LN2, store ----------
            fin = tc.tile_pool(name="fin_ps", bufs=2, space="PSUM")
            fin_ps = fin.__enter__()
            xrows = [singles.tile([128, H], F32, name=f"xrows{i}")
                     for i in range(NBLK)]
            for i in range(NBLK):
                ps = fin_ps.tile([128, H], F32, tag="ops")
                for k in range(HCH):
                    nc.tensor.matmul(out=ps[:], lhsT=ctxT[k][:, i * 128:(i + 1) * 128],
                                     rhs=wct[k][:, O_O:O_O + H],
                                     start=(k == 0), stop=(k == HCH - 1))
                z = singles.tile([128, H], F32, name=f"z{i}")
                nc.vector.tensor_tensor(z[:], in0=ps[:], in1=bct[:, B_O:B_O + H],
                                        op=mybir.AluOpType.add)
                nc.gpsimd.tensor_tensor(z[:], in0=z[:], in1=hr[:, i * H:(i + 1) * H],
                                        op=mybir.AluOpType.add)
                _layernorm_rows(nc, singles, z, xrows[i][:],
                                bct[:, B_G1:B_G1 + H], bct[:, B_BE1:B_BE1 + H],
                                i, "ln1", epst)
            xT = [singles.tile([128, RPC], BF, name=f"xT{j}") for j in range(HCH)]
            for i in range(NBLK):
                for j in range(HCH):
                    tp = fin_ps.tile([128, 128], F32, tag="tp")
                    nc.tensor.transpose(out=tp[:], in_=xrows[i][:, j * 128:(j + 1) * 128],
                                        identity=ident[:])
                    nc.vector.tensor_copy(out=xT[j][:, i * 128:(i + 1) * 128], in_=tp[:])

            y1T = [singles.tile([128, RPC], BF, name=f"y1T{j}") for j in range(4)]
            for j in range(4):
                ps = fin_ps.tile([128, RPC], F32, tag="y1ps")
                for k in range(HCH):
                    nc.tensor.matmul(out=ps[:], lhsT=wct[k][:, O_1 + j * 128:O_1 + (j + 1) * 128],
                                     rhs=xT[k][:], start=(k == 0), stop=(k == HCH - 1))
                nc.scalar.activation(out=y1T[j][:], in_=ps[:], func=AF.Gelu,
                                     bias=bct[:, B_1 + j:B_1 + j + 1], scale=1.0)

            W2CH = [(0, O_2A), (1, O_2A), (0, O_2B), (1, O_2B)]
            o_all = singles.tile([128, NBLK * H], F32, name="o_all")
            for i in range(NBLK):
                ps = fin_ps.tile([128, H], F32, tag="o2ps")
                for k2 in range(4):
                    kk, oo = W2CH[k2]
                    nc.tensor.matmul(out=ps[:], lhsT=y1T[k2][:, i * 128:(i + 1) * 128],
                                     rhs=wct[kk][:, oo:oo + H],
                                     start=(k2 == 0), stop=(k2 == 3))
                z = singles.tile([128, H], F32, name=f"z2{i}")
                nc.vector.tensor_tensor(z[:], in0=ps[:], in1=bct[:, B_2:B_2 + H],
                                        op=mybir.AluOpType.add)
                nc.gpsimd.tensor_tensor(z[:], in0=z[:], in1=xrows[i][:],
                                        op=mybir.AluOpType.add)
                _layernorm_rows(nc, singles, z, o_all[:, i * H:(i + 1) * H],
                                bct[:, B_G2:B_G2 + H], bct[:, B_BE2:B_BE2 + H],
                                i, "ln2", epst)
                nc.sync.dma_start(
                    out=out_d[i * 128:(i + 1) * 128, :],
                    in_=o_all[:, i * H:(i + 1) * H])
            fin.__exit__(None, None, None)
        big.__exit__(None, None, None)

    nc.compile()
    return nc


def _prep_graph(edge_index):
    src = np.asarray(edge_index[0]).astype(np.int64)
    dst = np.asarray(edge_index[1]).astype(np.int64)
    counts = np.bincount(src * N + dst, minlength=N * N).astype(np.float32)
    counts = counts.reshape(N, N)
    deg = counts.sum(axis=0)
    cn = counts / np.maximum(deg, 1.0)[None, :]
    m = (deg > 0).astype(np.float32)
    return cn, m


def _build_in_maps(inputs):
    bf = ml_dtypes.bfloat16
    h = np.asarray(inputs["h"], np.float32)
    cn, m = _prep_graph(inputs["edge_index"])
    W = {k: np.asarray(inputs[k], np.float32) for k in
         ("W_src", "W_tgt", "Wq", "Wk", "Wv", "Wo", "W1", "W2")}
    B = {k: np.asarray(inputs[k], np.float32) for k in
         ("b_src", "b_tgt", "bq", "bk", "bv", "bo", "b1", "b2",
          "g1", "be1", "g2", "be2")}

    wcat = np.concatenate(
        [W["W_src"], W["W_tgt"], W["Wq"], W["Wk"], W["Wv"], W["Wo"],
         W["W1"], W["W2"][0:256, :], W["W2"][256:512, :]], axis=1)
    assert wcat.shape == (H, WCAT_COLS)

    bo_eff = B["bo"] + B["bv"] @ W["Wo"]

    def bcast(v):
        return np.tile(v[None, :], (128, 1)).astype(np.float32)

    bcat = np.concatenate(
        [bcast(bo_eff), bcast(B["b2"]), bcast(B["g1"]), bcast(B["be1"]),
         bcast(B["g2"]), bcast(B["be2"]),
         np.ascontiguousarray(B["bq"].reshape(HCH, 128).T.astype(np.float32)),
         np.ascontiguousarray(B["b1"].reshape(4, 128).T.astype(np.float32))],
        axis=1)
    assert bcat.shape == (128, BCAT_COLS)

    bst = (B["b_src"] + B["b_tgt"]).astype(np.float32)
    hT = np.ascontiguousarray(h.T)

    common = {
        "h_bf": h.astype(bf),
        "wcat": np.ascontiguousarray(wcat).astype(bf),
        "bcat": np.ascontiguousarray(bcat),
    }
    in_maps = []
    for c in range(NCORES):
        sl = slice(c * RPC, (c + 1) * RPC)
        mc = m[sl]
        hTo = hT[:, sl]
        hcat = np.concatenate([hTo, hTo * mc[None, :]], axis=0)
        mm_ = dict(common)
        mm_["cn"] = np.ascontiguousarray(cn[:, sl]).astype(bf)
        mm_["hcat"] = np.ascontiguousarray(hcat).astype(bf)
        mm_["h_rows"] = np.ascontiguousarray(h[sl, :])
        mm_["sm1"] = np.ascontiguousarray(
            np.concatenate([bst, mc])[None, :]).astype(bf)
        in_maps.append(mm_)
    return in_maps


def kernel(**inputs):
    if "prog" not in _CACHE:
        _CACHE["prog"] = _build_program()
    nc = _CACHE["prog"]
    in_maps = _build_in_maps(inputs)
    res = run_bass_kernel_spmd(nc, in_maps, list(range(NCORES)))
    return np.concatenate([res.results[c]["out"] for c in range(NCORES)], axis=0)


if __name__ == "__main__":
    import reference
    inp = reference.setup_inputs()
    outp = kernel(**{k: np.asarray(v) for k, v in inp.items()})
    print("kernel out:", outp.shape, outp.dtype)
